# revision 9
# baseline (speedup 1.0000x reference)
"""DigitCaps dynamic-routing kernel for 8 Trainium2 NeuronCores.

Math (reference):
    u_hat[b,c,u,k] = sum_i W[c,u,k,i] * x[b,i,c]          (B=32, I=16, C=8192, U=32, K=16)
    b_ij = 0
    repeat 3x:
        c_ij  = softmax(b_ij, axis=c)
        s     = sum_c c_ij[c,u] * u_hat[b,c,u,k]
        v     = squash(s)    (norm over u, per (b,k))
        b_ij += mean_b <u_hat, v>
    return v

Strategy (v2): shard C across the 8 cores (C_LOC = 1024 each).  W is
converted to bf16 host-side, DMA'd from HBM exactly ONCE (16.8 MB/core)
and kept RESIDENT in SBUF (128 KiB/partition) in (c, i, u, k) layout.
Each routing iteration is then a single fused sweep over the 8 channel
tiles with no HBM traffic:

  per tile n:
    PE : VX_i[c,(u,k)] = sum_b x[b,i,c]/B * v[b,(u,k)]   (16 row-tiled MMs)
    ACT: evacuate VX psum -> SBUF bf16 (frees DVE, enables 2x mode)
    DVE: prod = VX * W_n                (bf16 2x)
         a[c,u] = sum_{i,k} prod       (log2 halving adds at 2x + small reduce)
         b_state += a ; wexp = exp(b)  (softmax numerator; per-c = tile-local)
         wW = W_n * wexp[c,u]          (bf16)
    PE : s_part += sum_{c,i} xt_i[c,b] * wW[c,(u,k)]     (16 MMs, psum accum)
  One AllReduce per iteration of (s_part || Z_part) (~68 KB); softmax
  max-subtraction is skipped (b_ij stays within [-0.6,0.6]).  The squash
  is computed redundantly on every core; v replicated to 128 partitions.

t=0 needs no weighting (c_ij uniform): the W DMA sweep + plain s-matmuls.
"""

import contextlib

import numpy as np
import concourse.bass as bass
import concourse.bacc as bacc
import concourse.tile as tile
import concourse.mybir as mybir
from concourse.bass_utils import run_bass_kernel_spmd

B, I, C, U, K = 32, 16, 8192, 32, 16
UK = U * K
KI = K * I
IUK = I * U * K
N_CORES = 8
C_LOC = C // N_CORES
NT = C_LOC // 128
NUM_ITERS = 3
SZW = UK + U  # s || z AllReduce payload width

f32 = mybir.dt.float32
bf16 = mybir.dt.bfloat16
MUL = mybir.AluOpType.mult
ADD = mybir.AluOpType.add
Exp = mybir.ActivationFunctionType.Exp

_CACHE = {}


def _squash(nc, small, s_n, out_bf, out_f32, tag_sfx):
    """v = mag/(1+mag_sq) * s, norm over u per (b,k); writes out_bf or out_f32."""
    sq = small.tile([128, UK], f32, tag="sq" + tag_sfx)
    nc.vector.tensor_tensor(out=sq[:], in0=s_n[:], in1=s_n[:], op=MUL)
    mag_sq = small.tile([128, K], f32, tag="mag_sq" + tag_sfx)
    nc.vector.tensor_reduce(
        out=mag_sq[:],
        in_=sq[:].rearrange("b (u k) -> b k u", u=U),
        axis=mybir.AxisListType.X,
        op=ADD,
    )
    mag = small.tile([128, K], f32, tag="mag" + tag_sfx)
    nc.scalar.sqrt(mag[:], mag_sq[:])
    den = small.tile([128, K], f32, tag="den" + tag_sfx)
    nc.vector.tensor_scalar_add(out=den[:], in0=mag_sq[:], scalar1=1.0)
    rden = small.tile([128, K], f32, tag="rden" + tag_sfx)
    nc.vector.reciprocal(rden[:], den[:])
    fac = small.tile([128, K], f32, tag="fac" + tag_sfx)
    nc.vector.tensor_tensor(out=fac[:], in0=mag[:], in1=rden[:], op=MUL)
    if out_bf is not None:
        nc.vector.tensor_tensor(
            out=out_bf[:].rearrange("b (u k) -> b k u", u=U),
            in0=s_n[:].rearrange("b (u k) -> b k u", u=U),
            in1=fac[:].broadcast_to([128, K, U]),
            op=MUL,
        )
    else:
        nc.vector.tensor_tensor(
            out=out_f32[:].rearrange("b (u k) -> b k u", u=U),
            in0=s_n[:B, :].rearrange("b (u k) -> b k u", u=U),
            in1=fac[:B, :].broadcast_to([B, K, U]),
            op=MUL,
        )


def _body(nc, w_in, xn_in, xt_in, v_out, fake_cc=False, repeat=1, ig=2):
    IG = ig
    NG = I // IG
    tc_pools = [
        ("wres", dict(bufs=1)),
        ("xpool", dict(bufs=1)),
        ("spool", dict(bufs=1)),
        ("vxpool", dict(bufs=2)),
        ("wwpool", dict(bufs=2)),
        ("small", dict(bufs=1)),
        ("wukpool", dict(bufs=1)),
        ("pvx", dict(bufs=3, space="PSUM")),
        ("pacc", dict(bufs=1, space="PSUM")),
        ("dram", dict(bufs=1, space="DRAM")),
    ]
    with tile.TileContext(nc) as tc, contextlib.ExitStack() as stack:
        pools = [stack.enter_context(tc.tile_pool(name=n, **kw)) for n, kw in tc_pools]
        wres, xpool, spool, vxpool, wwpool, small, wukpool, pvx, pacc, dram = pools

        # ---- persistent tiles ----
        wr = wres.tile([128, NT * IUK], bf16)  # resident W, (n, i, u, k)
        wr4 = wr[:].rearrange("c (n i z) -> c n i z", n=NT, i=I)
        xn = xpool.tile([128, 4 * C_LOC], bf16)
        nc.sync.dma_start(xn[:], xn_in[:])
        xn3 = xn[:].rearrange("p (il c) -> p il c", il=4)
        xt = xpool.tile([128, NT * I * B], bf16)
        nc.sync.dma_start(xt[:], xt_in[:])
        xt4 = xt[:].rearrange("c (n i b) -> c n i b", n=NT, i=I)
        ones_bf = xpool.tile([128, B], bf16)
        nc.vector.memset(ones_bf[:], 1.0)
        b_state = spool.tile([128, NT * U], f32)
        wexp_state = spool.tile([128, NT * U], bf16)
        vrep = spool.tile([128, UK], bf16)

        for rep in range(repeat):
          nc.vector.memset(b_state[:], 0.0)
          for t in range(NUM_ITERS):
            ps_s = pacc.tile([B, UK], f32, tag="ps_s")
            if t == 0:
                # ---- W DMA sweep + unweighted s-matmuls ----
                for n in range(NT):
                    nc.sync.dma_start(
                        wr[:, n * IUK : (n + 1) * IUK],
                        w_in[bass.ts(n, 128), :],
                    )
                    for i in range(I):
                        nc.tensor.matmul(
                            ps_s[:],
                            xt4[:, n, i, :],
                            wr4[:, n, i, :],
                            start=(n == 0 and i == 0),
                            stop=(n == NT - 1 and i == I - 1),
                        )
            else:
                ps_z = pacc.tile([B, U], f32, tag="ps_z")
                for n in range(NT):
                    vx = vxpool.tile([128, IUK], bf16, tag="vx")
                    vx3 = vx[:].rearrange("c (i z) -> c i z", i=I)
                    for g in range(NG):
                        pv = pvx.tile([128, IG * UK], f32, tag="pv")
                        for j in range(IG):
                            i = g * IG + j
                            ih = i % 4
                            nc.tensor.matmul(
                                pv[:, bass.ts(j, UK)],
                                xn3[32 * ih : 32 * (ih + 1), i // 4, bass.ts(n, 128)],
                                vrep[32 * ih : 32 * (ih + 1), :],
                                start=True,
                                stop=True,
                                tile_position=(32 * ih, 0),
                            )
                        # ACT evacuates psum -> SBUF bf16
                        nc.scalar.copy(vx3[:, IG * g : IG * (g + 1)], pv[:])
                    # prod = VX * W (in place, bf16)
                    nc.vector.tensor_tensor(
                        out=vx[:], in0=vx[:], in1=wr[:, n * IUK : (n + 1) * IUK], op=MUL
                    )
                    # a[c,u] = sum_{i,k} prod : halve i 16->1, then reduce k
                    vxi = vx[:].rearrange("c (i z) -> c i z", i=I)
                    for h in (8, 4, 2, 1):
                        nc.vector.tensor_tensor(
                            out=vxi[:, 0:h],
                            in0=vxi[:, 0:h],
                            in1=vxi[:, h : 2 * h],
                            op=ADD,
                        )
                    a_red = small.tile([128, U], f32, tag="a_red")
                    nc.vector.tensor_reduce(
                        out=a_red[:],
                        in_=vx[:, 0:UK].rearrange("c (u k) -> c u k", u=U),
                        axis=mybir.AxisListType.X,
                        op=ADD,
                    )
                    b_slice = b_state[:, bass.ts(n, U)]
                    nc.vector.tensor_tensor(
                        out=b_slice, in0=b_slice, in1=a_red[:], op=ADD
                    )
                    wexp = wexp_state[:, bass.ts(n, U)]
                    nc.scalar.activation(wexp, b_slice, Exp)
                    # wexp broadcast over k, then over i; wW = W * wexp
                    wuk = wukpool.tile([128, UK], bf16, tag="wuk")
                    nc.vector.tensor_copy(
                        out=wuk[:].rearrange("c (u k) -> c u k", u=U),
                        in_=wexp.broadcast_to([128, U, K]),
                    )
                    # z partial: ps_z += ones^T wexp (per tile, accumulated)
                    nc.tensor.matmul(
                        ps_z[:],
                        ones_bf[:, :U],
                        wexp,
                        start=(n == 0),
                        stop=(n == NT - 1),
                    )
                    # wW in half-tiles so s-matmuls start before the full tile
                    IH = I // 2
                    for h in range(2):
                        ww = wwpool.tile([128, IH * UK], bf16, tag="ww")
                        ww3 = ww[:].rearrange("c (i z) -> c i z", i=IH)
                        nc.vector.tensor_tensor(
                            out=ww3[:],
                            in0=wr4[:, n, h * IH : (h + 1) * IH],
                            in1=wuk[:]
                            .rearrange("c (o z) -> c o z", o=1)
                            .broadcast_to([128, IH, UK]),
                            op=MUL,
                        )
                        for ii in range(IH):
                            i = h * IH + ii
                            nc.tensor.matmul(
                                ps_s[:],
                                xt4[:, n, i, :],
                                ww3[:, ii],
                                start=(n == 0 and i == 0),
                                stop=(n == NT - 1 and i == I - 1),
                            )

            # ---- AllReduce s (and Z for t>0), then squash ----
            sz = small.tile([B, SZW], f32, tag="sz")
            nc.vector.tensor_copy(out=sz[:, :UK], in_=ps_s[:])
            if t > 0:
                nc.vector.tensor_copy(out=sz[:, UK:], in_=ps_z[:])
            else:
                nc.vector.memset(sz[:, UK:], 1.0)
            cc_in = dram.tile([B, SZW], f32, tag="cc_in")
            cc_out = dram.tile([B, SZW], f32, tag="cc_out")
            nc.sync.dma_start(cc_in[:], sz[:])
            if fake_cc:
                nc.sync.dma_start(cc_out[:], cc_in[:])
            else:
                nc.gpsimd.collective_compute(
                    "AllReduce",
                    ADD,
                    replica_groups=[list(range(N_CORES))],
                    ins=[cc_in.opt()],
                    outs=[cc_out.opt()],
                )
            sz_all = small.tile([128, SZW], f32, tag="sz_all")
            for g in range(4):
                nc.sync.dma_start(sz_all[32 * g : 32 * (g + 1), :], cc_out[:])

            s_n = small.tile([128, UK], f32, tag="s_n")
            if t == 0:
                nc.scalar.mul(s_n[:], sz_all[:, :UK], 1.0 / C)
            else:
                rz = small.tile([128, U], f32, tag="rz")
                nc.vector.reciprocal(rz[:], sz_all[:, UK:])
                nc.vector.tensor_tensor(
                    out=s_n[:].rearrange("b (u k) -> b u k", u=U),
                    in0=sz_all[:, :UK].rearrange("b (u k) -> b u k", u=U),
                    in1=rz[:].broadcast_to([128, U, K]),
                    op=MUL,
                )
            if t < NUM_ITERS - 1:
                _squash(nc, small, s_n, vrep, None, tag_sfx="")
            else:
                v_t = small.tile([B, UK], f32, tag="v_t")
                _squash(nc, small, s_n, None, v_t, tag_sfx="")
                nc.sync.dma_start(v_out[:], v_t[:])


def _build():
    if "nc" in _CACHE:
        return _CACHE["nc"]
    nc = bacc.Bacc(
        "TRN2", target_bir_lowering=False, debug=False, num_devices=N_CORES
    )
    w_in = nc.dram_tensor("w", [C_LOC, IUK], bf16, kind="ExternalInput").ap()
    xn_in = nc.dram_tensor("xn", [128, 4 * C_LOC], bf16, kind="ExternalInput").ap()
    xt_in = nc.dram_tensor("xt", [128, NT * I * B], bf16, kind="ExternalInput").ap()
    v_out = nc.dram_tensor("v_out", [B, UK], f32, kind="ExternalOutput").ap()
    _body(nc, w_in, xn_in, xt_in, v_out)
    nc.compile()
    _CACHE["nc"] = nc
    return nc


def _prep_inputs(x, W):
    """Shard FULL inputs into per-core DMA-friendly bf16 layouts."""
    import ml_dtypes

    bf = ml_dtypes.bfloat16
    x = np.asarray(x, dtype=np.float32)
    W = np.asarray(W, dtype=np.float32)
    in_maps = []
    for r in range(N_CORES):
        # W slice (C_LOC, U, K, I) -> (c, i, u, k) -> [C_LOC, I*U*K] bf16
        w_r = np.ascontiguousarray(
            W[r * C_LOC : (r + 1) * C_LOC].transpose(0, 3, 1, 2)
        ).reshape(C_LOC, IUK).astype(bf)
        xs = x[:, :, r * C_LOC : (r + 1) * C_LOC]  # [B, I, C_LOC] view
        # xn[32*(i%4) + b, (i//4)*C_LOC + c] = xs[b, i, c] / B
        xn_r = (
            np.ascontiguousarray(
                xs.transpose(1, 0, 2).reshape(4, 4, B, C_LOC).transpose(1, 2, 0, 3)
            ).reshape(128, 4 * C_LOC)
            * np.float32(1.0 / B)
        ).astype(bf)
        # xt[cc, (tile, i, b)] = xs[b, i, tile*128 + cc]
        xt_r = np.ascontiguousarray(
            xs.reshape(B, I, NT, 128).transpose(3, 2, 1, 0)
        ).reshape(128, NT * I * B).astype(bf)
        in_maps.append({"w": w_r, "xn": xn_r, "xt": xt_r})
    return in_maps


def kernel(x, W):
    nc = _build()
    in_maps = _prep_inputs(x, W)
    # The shared device occasionally wedges (NRT_EXEC_UNIT_UNRECOVERABLE)
    # and recovers on the next attempt — retry once before giving up.
    try:
        res = run_bass_kernel_spmd(nc, in_maps, core_ids=list(range(N_CORES)))
    except Exception:
        import time as _time

        _time.sleep(15)
        res = run_bass_kernel_spmd(nc, in_maps, core_ids=list(range(N_CORES)))
    v = res.results[0]["v_out"]
    return v.reshape(B, U, K, 1).astype(np.float32)


def make_runner(nc, in_maps):
    """Device-resident repeat runner (timing infrastructure for test.py)."""
    import jax
    from concourse import bass2jax
    from concourse.bass2jax import _bass_exec_p, install_neuronx_cc_hook
    from jax.experimental.shard_map import shard_map
    from jax.sharding import Mesh, PartitionSpec, NamedSharding

    install_neuronx_cc_hook()
    n_cores = len(in_maps)
    partition_name = nc.partition_id_tensor.name if nc.partition_id_tensor else None
    in_names, out_names, out_avals, zero_outs = [], [], [], []
    for alloc in nc.m.functions[0].allocations:
        if not isinstance(alloc, mybir.MemoryLocationSet):
            continue
        name = alloc.memorylocations[0].name
        if alloc.kind == "ExternalInput":
            if name != partition_name:
                in_names.append(name)
        elif alloc.kind == "ExternalOutput":
            out_names.append(name)
            shape = tuple(alloc.tensor_shape)
            dtype = mybir.dt.np(alloc.dtype)
            out_avals.append(jax.core.ShapedArray(shape, dtype))
            zero_outs.append(np.zeros(shape, dtype))
    n_params = len(in_names)
    n_outs = len(out_avals)
    all_in_names = list(in_names) + out_names
    if partition_name is not None:
        all_in_names.append(partition_name)

    def _body(*args):
        operands = list(args)
        if partition_name is not None:
            operands.append(bass2jax.partition_id_tensor())
        outs = _bass_exec_p.bind(
            *operands,
            out_avals=tuple(out_avals),
            in_names=tuple(all_in_names),
            out_names=tuple(out_names),
            lowering_input_output_aliases=(),
            sim_require_finite=True,
            sim_require_nnan=True,
            nc=nc,
        )
        return tuple(outs)

    devices = jax.devices()[:n_cores]
    mesh = Mesh(np.asarray(devices), ("core",))
    in_specs = (PartitionSpec("core"),) * (n_params + n_outs)
    out_specs = (PartitionSpec("core"),) * len(out_names)
    donate = tuple(range(n_params, n_params + n_outs))
    sharded = jax.jit(
        shard_map(
            _body, mesh=mesh, in_specs=in_specs, out_specs=out_specs, check_rep=False
        ),
        donate_argnums=donate,
        keep_unused=True,
    )
    sh = NamedSharding(mesh, PartitionSpec("core"))
    concat_in = [
        jax.device_put(
            np.concatenate([np.asarray(in_maps[c][nm]) for c in range(n_cores)], 0),
            sh,
        )
        for nm in in_names
    ]
    for a in concat_in:
        a.block_until_ready()

    def run(n_iter=1):
        outs = None
        for _ in range(n_iter):
            zeros = [
                np.zeros((n_cores * z.shape[0], *z.shape[1:]), z.dtype)
                for z in zero_outs
            ]
            outs = sharded(*concat_in, *zeros)
        for o in outs:
            o.block_until_ready()
        return outs

    return run


# revision 10
# speedup vs baseline: 1.0407x; 1.0407x over previous
"""DigitCaps dynamic-routing kernel for 8 Trainium2 NeuronCores.

Math (reference):
    u_hat[b,c,u,k] = sum_i W[c,u,k,i] * x[b,i,c]          (B=32, I=16, C=8192, U=32, K=16)
    b_ij = 0
    repeat 3x:
        c_ij  = softmax(b_ij, axis=c)
        s     = sum_c c_ij[c,u] * u_hat[b,c,u,k]
        v     = squash(s)    (norm over u, per (b,k))
        b_ij += mean_b <u_hat, v>
    return v

Strategy (v2): shard C across the 8 cores (C_LOC = 1024 each).  W is
converted to bf16 host-side, DMA'd from HBM exactly ONCE (16.8 MB/core)
and kept RESIDENT in SBUF (128 KiB/partition) in (c, i, u, k) layout.
Each routing iteration is then a single fused sweep over the 8 channel
tiles with no HBM traffic:

  per tile n:
    PE : VX_i[c,(u,k)] = sum_b x[b,i,c]/B * v[b,(u,k)]   (16 row-tiled MMs)
    ACT: evacuate VX psum -> SBUF bf16 (frees DVE, enables 2x mode)
    DVE: prod = VX * W_n                (bf16 2x)
         a[c,u] = sum_{i,k} prod       (log2 halving adds at 2x + small reduce)
         b_state += a ; wexp = exp(b)  (softmax numerator; per-c = tile-local)
         wW = W_n * wexp[c,u]          (bf16)
    PE : s_part += sum_{c,i} xt_i[c,b] * wW[c,(u,k)]     (16 MMs, psum accum)
  One AllReduce per iteration of (s_part || Z_part) (~68 KB); softmax
  max-subtraction is skipped (b_ij stays within [-0.6,0.6]).  The squash
  is computed redundantly on every core; v replicated to 128 partitions.

t=0 needs no weighting (c_ij uniform): the W DMA sweep + plain s-matmuls.
"""

import contextlib

import numpy as np
import concourse.bass as bass
import concourse.bacc as bacc
import concourse.tile as tile
import concourse.mybir as mybir
from concourse.bass_utils import run_bass_kernel_spmd

B, I, C, U, K = 32, 16, 8192, 32, 16
UK = U * K
KI = K * I
IUK = I * U * K
N_CORES = 8
C_LOC = C // N_CORES
NT = C_LOC // 128
NUM_ITERS = 3
SZW = UK + U  # s || z AllReduce payload width

f32 = mybir.dt.float32
bf16 = mybir.dt.bfloat16
MUL = mybir.AluOpType.mult
ADD = mybir.AluOpType.add
Exp = mybir.ActivationFunctionType.Exp

_CACHE = {}


def _squash(nc, small, s_n, out_bf, out_f32, tag_sfx):
    """v = mag/(1+mag_sq) * s, norm over u per (b,k); writes out_bf or out_f32."""
    sq = small.tile([128, UK], f32, tag="sq" + tag_sfx)
    nc.vector.tensor_tensor(out=sq[:], in0=s_n[:], in1=s_n[:], op=MUL)
    mag_sq = small.tile([128, K], f32, tag="mag_sq" + tag_sfx)
    nc.vector.tensor_reduce(
        out=mag_sq[:],
        in_=sq[:].rearrange("b (u k) -> b k u", u=U),
        axis=mybir.AxisListType.X,
        op=ADD,
    )
    mag = small.tile([128, K], f32, tag="mag" + tag_sfx)
    nc.scalar.sqrt(mag[:], mag_sq[:])
    den = small.tile([128, K], f32, tag="den" + tag_sfx)
    nc.vector.tensor_scalar_add(out=den[:], in0=mag_sq[:], scalar1=1.0)
    rden = small.tile([128, K], f32, tag="rden" + tag_sfx)
    nc.vector.reciprocal(rden[:], den[:])
    fac = small.tile([128, K], f32, tag="fac" + tag_sfx)
    nc.vector.tensor_tensor(out=fac[:], in0=mag[:], in1=rden[:], op=MUL)
    if out_bf is not None:
        nc.vector.tensor_tensor(
            out=out_bf[:].rearrange("b (u k) -> b k u", u=U),
            in0=s_n[:].rearrange("b (u k) -> b k u", u=U),
            in1=fac[:].broadcast_to([128, K, U]),
            op=MUL,
        )
    else:
        nc.vector.tensor_tensor(
            out=out_f32[:].rearrange("b (u k) -> b k u", u=U),
            in0=s_n[:B, :].rearrange("b (u k) -> b k u", u=U),
            in1=fac[:B, :].broadcast_to([B, K, U]),
            op=MUL,
        )


def _body(nc, w_in, xn_in, xt_in, v_out, fake_cc=False, repeat=1, ig=2):
    IG = ig
    NG = I // IG
    tc_pools = [
        ("wres", dict(bufs=1)),
        ("xpool", dict(bufs=1)),
        ("spool", dict(bufs=1)),
        ("vxpool", dict(bufs=2)),
        ("wwpool", dict(bufs=2)),
        ("small", dict(bufs=1)),
        ("wukpool", dict(bufs=1)),
        ("pvx", dict(bufs=3, space="PSUM")),
        ("pacc", dict(bufs=1, space="PSUM")),
        ("dram", dict(bufs=1, space="DRAM")),
    ]
    with tile.TileContext(nc) as tc, contextlib.ExitStack() as stack:
        pools = [stack.enter_context(tc.tile_pool(name=n, **kw)) for n, kw in tc_pools]
        wres, xpool, spool, vxpool, wwpool, small, wukpool, pvx, pacc, dram = pools

        # ---- persistent tiles ----
        wr = wres.tile([128, NT * IUK], bf16)  # resident W, (n, i, u, k)
        wr4 = wr[:].rearrange("c (n i z) -> c n i z", n=NT, i=I)
        xn = xpool.tile([128, 4 * C_LOC], bf16)
        nc.sync.dma_start(xn[:], xn_in[:])
        xn3 = xn[:].rearrange("p (il c) -> p il c", il=4)
        xt = xpool.tile([128, NT * I * B], bf16)
        nc.sync.dma_start(xt[:], xt_in[:])
        xt4 = xt[:].rearrange("c (n i b) -> c n i b", n=NT, i=I)
        ones_bf = xpool.tile([128, B], bf16)
        nc.vector.memset(ones_bf[:], 1.0)
        b_state = spool.tile([128, NT * U], f32)
        wexp_state = spool.tile([128, NT * U], bf16)
        vrep = spool.tile([128, UK], bf16)

        for rep in range(repeat):
          nc.vector.memset(b_state[:], 0.0)
          for t in range(NUM_ITERS):
            ps_s = pacc.tile([B, UK], f32, tag="ps_s")
            if t == 0:
                # ---- W DMA sweep + unweighted s-matmuls ----
                for n in range(NT):
                    nc.sync.dma_start(
                        wr[:, n * IUK : (n + 1) * IUK],
                        w_in[bass.ts(n, 128), :],
                    )
                    for i in range(I):
                        nc.tensor.matmul(
                            ps_s[:],
                            xt4[:, n, i, :],
                            wr4[:, n, i, :],
                            start=(n == 0 and i == 0),
                            stop=(n == NT - 1 and i == I - 1),
                        )
            else:
                ps_z = pacc.tile([B, U], f32, tag="ps_z")
                for n in range(NT):
                    vx = vxpool.tile([128, IUK], bf16, tag="vx")
                    vx3 = vx[:].rearrange("c (i z) -> c i z", i=I)
                    for g in range(NG):
                        pv = pvx.tile([128, IG * UK], f32, tag="pv")
                        for j in range(IG):
                            i = g * IG + j
                            ih = i % 4
                            nc.tensor.matmul(
                                pv[:, bass.ts(j, UK)],
                                xn3[32 * ih : 32 * (ih + 1), i // 4, bass.ts(n, 128)],
                                vrep[32 * ih : 32 * (ih + 1), :],
                                start=True,
                                stop=True,
                                tile_position=(32 * ih, 0),
                            )
                        # ACT evacuates psum -> SBUF bf16
                        nc.scalar.copy(vx3[:, IG * g : IG * (g + 1)], pv[:])
                    # prod = VX * W (in place, bf16; halves so the first starts
                    # as soon as the first 4 psum groups are evacuated)
                    HF = IUK // 2
                    for h in range(2):
                        nc.vector.tensor_tensor(
                            out=vx[:, h * HF : (h + 1) * HF],
                            in0=vx[:, h * HF : (h + 1) * HF],
                            in1=wr[:, n * IUK + h * HF : n * IUK + (h + 1) * HF],
                            op=MUL,
                        )
                    # a[c,u] = sum_{i,k} prod : halve i 16->1, then reduce k
                    vxi = vx[:].rearrange("c (i z) -> c i z", i=I)
                    for h in (8, 4, 2, 1):
                        nc.vector.tensor_tensor(
                            out=vxi[:, 0:h],
                            in0=vxi[:, 0:h],
                            in1=vxi[:, h : 2 * h],
                            op=ADD,
                        )
                    a_red = small.tile([128, U], f32, tag="a_red")
                    nc.vector.tensor_reduce(
                        out=a_red[:],
                        in_=vx[:, 0:UK].rearrange("c (u k) -> c u k", u=U),
                        axis=mybir.AxisListType.X,
                        op=ADD,
                    )
                    b_slice = b_state[:, bass.ts(n, U)]
                    nc.vector.tensor_tensor(
                        out=b_slice, in0=b_slice, in1=a_red[:], op=ADD
                    )
                    wexp = wexp_state[:, bass.ts(n, U)]
                    nc.scalar.activation(wexp, b_slice, Exp)
                    # wexp broadcast over k, then over i; wW = W * wexp
                    wuk = wukpool.tile([128, UK], bf16, tag="wuk")
                    nc.vector.tensor_copy(
                        out=wuk[:].rearrange("c (u k) -> c u k", u=U),
                        in_=wexp.broadcast_to([128, U, K]),
                    )
                    # z partial: ps_z += ones^T wexp (per tile, accumulated)
                    nc.tensor.matmul(
                        ps_z[:],
                        ones_bf[:, :U],
                        wexp,
                        start=(n == 0),
                        stop=(n == NT - 1),
                    )
                    # wW in half-tiles so s-matmuls start before the full tile
                    IH = I // 2
                    for h in range(2):
                        ww = wwpool.tile([128, IH * UK], bf16, tag="ww")
                        ww3 = ww[:].rearrange("c (i z) -> c i z", i=IH)
                        nc.vector.tensor_tensor(
                            out=ww3[:],
                            in0=wr4[:, n, h * IH : (h + 1) * IH],
                            in1=wuk[:]
                            .rearrange("c (o z) -> c o z", o=1)
                            .broadcast_to([128, IH, UK]),
                            op=MUL,
                        )
                        for ii in range(IH):
                            i = h * IH + ii
                            nc.tensor.matmul(
                                ps_s[:],
                                xt4[:, n, i, :],
                                ww3[:, ii],
                                start=(n == 0 and i == 0),
                                stop=(n == NT - 1 and i == I - 1),
                            )

            # ---- AllReduce s (and Z for t>0), then squash ----
            sz = small.tile([B, SZW], f32, tag="sz")
            nc.vector.tensor_copy(out=sz[:, :UK], in_=ps_s[:])
            if t > 0:
                nc.vector.tensor_copy(out=sz[:, UK:], in_=ps_z[:])
            else:
                nc.vector.memset(sz[:, UK:], 1.0)
            cc_in = dram.tile([B, SZW], f32, tag="cc_in")
            cc_out = dram.tile([B, SZW], f32, tag="cc_out")
            nc.sync.dma_start(cc_in[:], sz[:])
            if fake_cc:
                nc.sync.dma_start(cc_out[:], cc_in[:])
            else:
                nc.gpsimd.collective_compute(
                    "AllReduce",
                    ADD,
                    replica_groups=[list(range(N_CORES))],
                    ins=[cc_in.opt()],
                    outs=[cc_out.opt()],
                )
            sz_all = small.tile([128, SZW], f32, tag="sz_all")
            for g in range(4):
                nc.sync.dma_start(sz_all[32 * g : 32 * (g + 1), :], cc_out[:])

            s_n = small.tile([128, UK], f32, tag="s_n")
            if t == 0:
                nc.scalar.mul(s_n[:], sz_all[:, :UK], 1.0 / C)
            else:
                rz = small.tile([128, U], f32, tag="rz")
                nc.vector.reciprocal(rz[:], sz_all[:, UK:])
                nc.vector.tensor_tensor(
                    out=s_n[:].rearrange("b (u k) -> b u k", u=U),
                    in0=sz_all[:, :UK].rearrange("b (u k) -> b u k", u=U),
                    in1=rz[:].broadcast_to([128, U, K]),
                    op=MUL,
                )
            if t < NUM_ITERS - 1:
                _squash(nc, small, s_n, vrep, None, tag_sfx="")
            else:
                v_t = small.tile([B, UK], f32, tag="v_t")
                _squash(nc, small, s_n, None, v_t, tag_sfx="")
                nc.sync.dma_start(v_out[:], v_t[:])


def _build():
    if "nc" in _CACHE:
        return _CACHE["nc"]
    nc = bacc.Bacc(
        "TRN2", target_bir_lowering=False, debug=False, num_devices=N_CORES
    )
    w_in = nc.dram_tensor("w", [C_LOC, IUK], bf16, kind="ExternalInput").ap()
    xn_in = nc.dram_tensor("xn", [128, 4 * C_LOC], bf16, kind="ExternalInput").ap()
    xt_in = nc.dram_tensor("xt", [128, NT * I * B], bf16, kind="ExternalInput").ap()
    v_out = nc.dram_tensor("v_out", [B, UK], f32, kind="ExternalOutput").ap()
    _body(nc, w_in, xn_in, xt_in, v_out)
    nc.compile()
    _CACHE["nc"] = nc
    return nc


def _prep_inputs(x, W):
    """Shard FULL inputs into per-core DMA-friendly bf16 layouts."""
    import ml_dtypes

    bf = ml_dtypes.bfloat16
    x = np.asarray(x, dtype=np.float32)
    W = np.asarray(W, dtype=np.float32)
    in_maps = []
    for r in range(N_CORES):
        # W slice (C_LOC, U, K, I) -> (c, i, u, k) -> [C_LOC, I*U*K] bf16
        w_r = np.ascontiguousarray(
            W[r * C_LOC : (r + 1) * C_LOC].transpose(0, 3, 1, 2)
        ).reshape(C_LOC, IUK).astype(bf)
        xs = x[:, :, r * C_LOC : (r + 1) * C_LOC]  # [B, I, C_LOC] view
        # xn[32*(i%4) + b, (i//4)*C_LOC + c] = xs[b, i, c] / B
        xn_r = (
            np.ascontiguousarray(
                xs.transpose(1, 0, 2).reshape(4, 4, B, C_LOC).transpose(1, 2, 0, 3)
            ).reshape(128, 4 * C_LOC)
            * np.float32(1.0 / B)
        ).astype(bf)
        # xt[cc, (tile, i, b)] = xs[b, i, tile*128 + cc]
        xt_r = np.ascontiguousarray(
            xs.reshape(B, I, NT, 128).transpose(3, 2, 1, 0)
        ).reshape(128, NT * I * B).astype(bf)
        in_maps.append({"w": w_r, "xn": xn_r, "xt": xt_r})
    return in_maps


def kernel(x, W):
    nc = _build()
    in_maps = _prep_inputs(x, W)
    # The shared device occasionally wedges (NRT_EXEC_UNIT_UNRECOVERABLE)
    # and recovers on the next attempt — retry once before giving up.
    try:
        res = run_bass_kernel_spmd(nc, in_maps, core_ids=list(range(N_CORES)))
    except Exception:
        import time as _time

        _time.sleep(15)
        res = run_bass_kernel_spmd(nc, in_maps, core_ids=list(range(N_CORES)))
    v = res.results[0]["v_out"]
    return v.reshape(B, U, K, 1).astype(np.float32)


def make_runner(nc, in_maps):
    """Device-resident repeat runner (timing infrastructure for test.py)."""
    import jax
    from concourse import bass2jax
    from concourse.bass2jax import _bass_exec_p, install_neuronx_cc_hook
    from jax.experimental.shard_map import shard_map
    from jax.sharding import Mesh, PartitionSpec, NamedSharding

    install_neuronx_cc_hook()
    n_cores = len(in_maps)
    partition_name = nc.partition_id_tensor.name if nc.partition_id_tensor else None
    in_names, out_names, out_avals, zero_outs = [], [], [], []
    for alloc in nc.m.functions[0].allocations:
        if not isinstance(alloc, mybir.MemoryLocationSet):
            continue
        name = alloc.memorylocations[0].name
        if alloc.kind == "ExternalInput":
            if name != partition_name:
                in_names.append(name)
        elif alloc.kind == "ExternalOutput":
            out_names.append(name)
            shape = tuple(alloc.tensor_shape)
            dtype = mybir.dt.np(alloc.dtype)
            out_avals.append(jax.core.ShapedArray(shape, dtype))
            zero_outs.append(np.zeros(shape, dtype))
    n_params = len(in_names)
    n_outs = len(out_avals)
    all_in_names = list(in_names) + out_names
    if partition_name is not None:
        all_in_names.append(partition_name)

    def _body(*args):
        operands = list(args)
        if partition_name is not None:
            operands.append(bass2jax.partition_id_tensor())
        outs = _bass_exec_p.bind(
            *operands,
            out_avals=tuple(out_avals),
            in_names=tuple(all_in_names),
            out_names=tuple(out_names),
            lowering_input_output_aliases=(),
            sim_require_finite=True,
            sim_require_nnan=True,
            nc=nc,
        )
        return tuple(outs)

    devices = jax.devices()[:n_cores]
    mesh = Mesh(np.asarray(devices), ("core",))
    in_specs = (PartitionSpec("core"),) * (n_params + n_outs)
    out_specs = (PartitionSpec("core"),) * len(out_names)
    donate = tuple(range(n_params, n_params + n_outs))
    sharded = jax.jit(
        shard_map(
            _body, mesh=mesh, in_specs=in_specs, out_specs=out_specs, check_rep=False
        ),
        donate_argnums=donate,
        keep_unused=True,
    )
    sh = NamedSharding(mesh, PartitionSpec("core"))
    concat_in = [
        jax.device_put(
            np.concatenate([np.asarray(in_maps[c][nm]) for c in range(n_cores)], 0),
            sh,
        )
        for nm in in_names
    ]
    for a in concat_in:
        a.block_until_ready()

    def run(n_iter=1):
        outs = None
        for _ in range(n_iter):
            zeros = [
                np.zeros((n_cores * z.shape[0], *z.shape[1:]), z.dtype)
                for z in zero_outs
            ]
            outs = sharded(*concat_in, *zeros)
        for o in outs:
            o.block_until_ready()
        return outs

    return run
